# revision 28
# baseline (speedup 1.0000x reference)
"""GAT BasicAttentionBlock kernel for 8x Trainium2 NeuronCores.

Strategy (output-shard, v2): each core owns 1250 of the 10000 selected
output rows (index0).  Only nodes reachable from those rows matter
(~1.2k targets + ~16k sources per core).  Per core:

  node table order: [window-grouped targets (1280 rows) | sources sorted
  by per-core edge multiplicity desc].  A static row boundary B1 (mult
  of 512) splits the table so rows < B1 carry ~2/3 of the edges.

  phase A  stream x columns of the table nodes (bf16), h = relu(x@w1.T)
           feature-major on PE, then per 128-node subtile
           proj|s_src = h@w2 node-major; copy cols 0:136 (bf16) of each
           PSUM tile to SBUF and DMA full 512-byte rows to the HBM table.
           Emission of loop-1 work (s_trg/skip/one-hot masks) is
           interleaved into phase A's engine slack.
  gathers  per 128-target window, edges sorted by source row: slots
           [0,KLO) hold only sources < B1 and are gathered as soon as
           the lo part of the table is written (overlaps phase A);
           slots [KLO,EC) gather after the full table.
  loop 2   per window: scores = lrelu(s_src+s_trg) on ACT (alpha=.2),
           e = exp, weighted = e*proj, segment-sum via one-hot matmuls
           accumulated in PSUM [sum e*proj | sum e]; out = att/den +
           skip, ELU; windows finish staggered as hi-gathers land.
  final    dma_gather the 1250 output rows from the 1280-target table.

No collectives: cores are fully independent.  The softmax global max
subtraction cancels in att = exp/sum(exp) and is dropped.
"""

import os
import sys

for _p in ("/opt/trn_rl_repo",):
    if os.path.isdir(_p) and _p not in sys.path:
        sys.path.insert(0, _p)

import numpy as np
import ml_dtypes

# problem constants (hardcoded per contract)
N = 50000
E = 800000
K = 10000
IN = 256
H = 128
NH = 8
HD = 16
OC = NH * HD  # 128
CORES = 8
KC = K // CORES          # 1250 output rows per core
P = 128
W = 10                   # target windows of 128 -> 1280 target slots
TP = W * P               # padded target count per core
EPS = 1e-16

BF16 = ml_dtypes.bfloat16

LOFRAC = float(os.environ.get("KLOFRAC", "0.55"))


# ----------------------------------------------------------------------------
# host-side sharding / planning
# ----------------------------------------------------------------------------

def _wrap16(vals, reps=8):
    """int16 index layout for dma_gather: idx i at [i%16, i//16], the 16-row
    block replicated `reps` times down the partition axis."""
    L = vals.shape[0]
    assert L % 16 == 0
    w = vals.reshape(L // 16, 16).T.astype(np.int16)
    return np.tile(w, (reps, 1))


def _binpack(deg):
    """Assign targets (by degree desc) to W windows (<=128 each), balancing
    total degree.  Returns row index (w*128 + pos) per target."""
    U = len(deg)
    order = np.argsort(-deg, kind="stable")
    wdeg = np.zeros(W)
    wcnt = np.zeros(W, np.int64)
    row = np.zeros(U, np.int64)
    for u in order:
        cand = np.nonzero(wcnt < P)[0]
        wsel = cand[np.argmin(wdeg[cand])]
        row[u] = wsel * P + wcnt[wsel]
        wcnt[wsel] += 1
        wdeg[wsel] += deg[u]
    return row


def plan(x, adj0, index0):
    src_all = np.asarray(adj0[0], dtype=np.int64)
    trg_all = np.asarray(adj0[1], dtype=np.int64)
    idx0 = np.asarray(index0, dtype=np.int64)
    x = np.asarray(x, dtype=np.float32)

    pre = []
    npad_req = 512
    for c in range(CORES):
        ks = idx0[c * KC:(c + 1) * KC]
        tgt_u, inv_k = np.unique(ks, return_inverse=True)
        U_t = len(tgt_u)
        assert U_t <= TP
        lut = np.full(N, -1, np.int64)
        lut[tgt_u] = np.arange(U_t)
        tloc_all = lut[trg_all]
        sel = np.nonzero(tloc_all >= 0)[0]
        e_src = src_all[sel]
        e_tu = tloc_all[sel]
        deg = np.bincount(e_tu, minlength=U_t)
        trow = _binpack(deg)                       # tgt_u idx -> table row

        # source rows: targets keep their rows; extras sorted by edge count
        nrow = np.full(N, -1, np.int64)
        nrow[tgt_u] = trow
        is_extra = nrow[e_src] < 0
        ex_ids, ex_cnt_inv = np.unique(e_src[is_extra], return_inverse=True)
        ex_cnt = np.bincount(ex_cnt_inv)
        ex_order = np.argsort(-ex_cnt, kind="stable")
        extras = ex_ids[ex_order]
        nrow[extras] = TP + np.arange(len(extras))
        U_n = TP + len(extras)
        npad_req = max(npad_req, U_n)

        e_srow = nrow[e_src]                       # source table row per edge
        e_trow = trow[e_tu]                        # target table row per edge
        # node id per table row (for xT); pad rows -> x of node 0 (harmless)
        nodes = np.zeros(U_n, np.int64)
        nodes[trow] = tgt_u
        nodes[TP:] = extras
        pre.append((trow, inv_k, e_srow, e_trow, nodes, U_n))

    NPAD = ((npad_req + 511) // 512) * 512

    # shared lo-chunk boundary B1 (mult of 512): rows < B1 carry >= LOFRAC
    # of edges on every core
    b1_req = 512
    for c in range(CORES):
        _, _, e_srow, _, _, _ = pre[c]
        hist = np.bincount(e_srow // 512, minlength=NPAD // 512)
        cum = np.cumsum(hist) / len(e_srow)
        t = int(np.searchsorted(cum, LOFRAC)) + 1
        b1_req = max(b1_req, t * 512)
    B1 = min((b1_req + 1023) // 1024 * 1024, NPAD)

    # per-core, per-window edge packing: strict lo/hi slot segregation
    # (lo gathers read only tabLo -> they can fire during phase A)
    klm_req = 1
    khi_req = 1
    packed = []
    for c in range(CORES):
        trow, inv_k, e_srow, e_trow, nodes, U_n = pre[c]
        e_win = e_trow >> 7
        is_lo = e_srow < B1
        order = np.lexsort((~is_lo, e_win))
        e_srow = e_srow[order]
        e_trow = e_trow[order]
        e_win = e_win[order]
        nlo = np.bincount(e_win[e_srow < B1], minlength=W)
        nhi = np.bincount(e_win[e_srow >= B1], minlength=W)
        klm_req = max(klm_req, int(np.ceil(nlo.max() / P)))
        khi_req = max(khi_req, int(np.ceil(nhi.max() / P)))
        packed.append((trow, inv_k, e_srow, e_trow, e_win, nlo, nhi, nodes))

    KLO = klm_req
    KHI = khi_req
    EC = KLO + KHI
    cap = EC * P

    per_core = []
    for c in range(CORES):
        trow, inv_k, e_srow, e_trow, e_win, nlo, nhi, nodes = packed[c]
        # slot within window: lo edges at [0, nlo), hi at [KLO*P, KLO*P+nhi)
        cnt = nlo + nhi
        start = np.concatenate([[0], np.cumsum(cnt)[:-1]])
        within = np.arange(len(e_trow)) - start[e_win]
        is_hi = within >= nlo[e_win]
        within = within + is_hi * (KLO * P - nlo[e_win])
        slots = e_win * cap + within

        esrc_flat = np.zeros(W * cap, np.int64)
        # dummy hi slots point at tabHi row 0
        for w in range(W):
            esrc_flat[w * cap + KLO * P:(w + 1) * cap] = B1
        etcol_flat = np.full(W * cap, -1.0, np.float32)
        esrc_flat[slots] = e_srow
        etcol_flat[slots] = (e_trow - e_win * P).astype(np.float32)

        etcol = etcol_flat.reshape(W, EC, P).transpose(2, 0, 1).reshape(P, W * EC)
        etrow_b = etcol_flat.astype(BF16).reshape(1, W * cap)
        eidx_lo = np.concatenate(
            [_wrap16(esrc_flat[w * cap:w * cap + KLO * P]) for w in range(W)],
            axis=1)
        eidx_hi = np.concatenate(
            [_wrap16(esrc_flat[w * cap + KLO * P:(w + 1) * cap] - B1)
             for w in range(W)], axis=1)

        kvals = np.zeros(TP, np.int64)
        kvals[:KC] = trow[inv_k]
        kidx = _wrap16(kvals)

        xT = np.zeros((IN, NPAD), BF16)
        xT[:, :len(nodes)] = x[nodes].T
        # interleave the two 128-row halves chunk-wise: one DMA per chunk
        CW = 1024  # CH(=2) * 512
        assert NPAD % CW == 0
        xTi = np.empty((P, 2 * NPAD), BF16)
        for c in range(NPAD // CW):
            xTi[:, 2 * c * CW:2 * c * CW + CW] = xT[0:P, c * CW:(c + 1) * CW]
            xTi[:, 2 * c * CW + CW:2 * (c + 1) * CW] = \
                xT[P:IN, c * CW:(c + 1) * CW]

        iblob = np.concatenate([eidx_lo, eidx_hi, kidx], axis=1)
        per_core.append(dict(xTi=xTi, iblob=iblob,
                             etcol=etcol, etrow=etrow_b))
    return per_core, NPAD, EC, KLO, B1


def make_weights(w_in, b_in, w_proj, a_src, a_trg, w_skip):
    w_in = np.asarray(w_in, np.float32)
    b_in = np.asarray(b_in, np.float32)
    w_proj = np.asarray(w_proj, np.float32)
    a_src = np.asarray(a_src, np.float32).reshape(NH, HD)
    a_trg = np.asarray(a_trg, np.float32).reshape(NH, HD)
    w_skip = np.asarray(w_skip, np.float32)

    w1T = np.ascontiguousarray(w_in.T).astype(BF16)        # [256,128]
    b1 = b_in.reshape(H, 1).astype(np.float32)
    # B_src[h, a] = sum_d w_proj[a*16+d, h] * a_src[a, d]
    wp3 = w_proj.reshape(NH, HD, H)
    B_src = np.einsum("adh,ad->ha", wp3, a_src).astype(np.float32)  # [128,8]
    B_trg = np.einsum("adh,ad->ha", wp3, a_trg).astype(BF16)
    w2 = np.zeros((H, 256), np.float32)  # cast to bf16 below
    w2[:, :OC] = w_proj.T
    w2[:, OC:OC + NH] = B_src
    wskT = np.ascontiguousarray(w_skip.T).astype(BF16)     # [128,128]
    iota4 = np.tile(np.arange(P, dtype=BF16)[None, :], (P, 2))
    iota_c = np.arange(P, dtype=np.float32).reshape(P, 1)
    bfblob = np.concatenate(
        [np.ascontiguousarray(w1T[0:P]), np.ascontiguousarray(w1T[P:IN]),
         w2.astype(BF16), wskT, B_trg, iota4], axis=1)  # [128, 776]
    return dict(bfblob=bfblob, b1=b1, iota_c=iota_c)


# ----------------------------------------------------------------------------
# bass kernel
# ----------------------------------------------------------------------------

_BUILD_CACHE = {}


def build(NPAD, EC, KLO, B1):
    key = (NPAD, EC, KLO, B1)
    if key in _BUILD_CACHE:
        return _BUILD_CACHE[key]

    import concourse.bacc as bacc
    import concourse.mybir as mybir
    import concourse.tile as tile

    dt = mybir.dt
    F32 = dt.float32
    F32R = dt.float32r
    I16 = dt.int16
    BF = dt.bfloat16
    AF = mybir.ActivationFunctionType
    OP = mybir.AluOpType

    NT = NPAD // 512
    cap = EC * P
    KHI = EC - KLO

    nc = bacc.Bacc("TRN2", target_bir_lowering=False)

    with tile.TileContext(nc) as tc:
        with tc.tile_pool(name="dram", bufs=1, space="DRAM") as dram:
            def din(name, shape, dtp):
                return dram.tile(shape, dtp, kind="ExternalInput", name=name,
                                 uniquify=False)

            NBF = H + H + 256 + OC + NH + 2 * P  # 904
            NI16 = W * KLO * 8 + W * KHI * 8 + TP // 16
            xTi = din("xTi", [P, 2 * NPAD], BF)
            bfblob = din("bfblob", [P, NBF], BF)
            fblob = din("fblob", [P, 2 + W * EC], F32)
            iblob = din("iblob", [P, NI16], I16)
            etrow = din("etrow", [1, W * cap], BF)

            tabLo = dram.tile([B1, 256], BF, kind="Internal", name="tabLo",
                              uniquify=False)
            tabHi = dram.tile([NPAD - B1, 256], BF, kind="Internal",
                              name="tabHi", uniquify=False)
            outT = dram.tile([TP, OC], BF, kind="Internal", name="outT",
                             uniquify=False)
            out = dram.tile([TP, OC], BF, kind="ExternalOutput", name="out",
                            uniquify=False)

        with tc.tile_pool(name="pers", bufs=1) as pers:
            bfb = pers.tile([P, NBF], BF)
            fb = pers.tile([P, 2 + W * EC], F32)
            ib = pers.tile([P, NI16], I16)
            hfmt = pers.tile([H, TP], BF)         # targets' h, feature-major
            strg = pers.tile([P, W * NH], BF)     # per-window s_trg  [t, 8]
            skips = pers.tile([P, W, OC], BF)     # per-window skip   [t, oc]
            st_sb = pers.tile([P, W, EC, NH], BF)   # s_trg per edge slot
            Mw = pers.tile([P, W * cap], BF)      # edge->target one-hot
            iotaL = pers.tile([P, P], BF)         # iota copy, gated mid-phase
            Glo = pers.tile([P, W, KLO, 256], BF)  # lo-gathered table rows
            etws = pers.tile([1, W * cap], BF)

            nc.sync.dma_start(etws[:], etrow[:])
            nc.sync.dma_start(fb[:], fblob[:])
            nc.sync.dma_start(bfb[:], bfblob[:])
            nc.sync.dma_start(ib[:], iblob[:])

            w1a = bfb[:, 0:H]
            w1b = bfb[:, H:2 * H]
            w2s = bfb[:, 2 * H:2 * H + 256]
            wsks = bfb[:, 2 * H + 256:2 * H + 256 + OC]
            btrgs = bfb[:, 2 * H + 256 + OC:2 * H + 256 + OC + NH]
            iota4s = bfb[:, 2 * H + 256 + OC + NH:NBF]
            b1s = fb[:, 0:1]
            iotac = fb[:, 1:2]
            etcols = fb[:, 2:2 + W * EC]
            eloidx = ib[:, 0:W * KLO * 8]
            ehiidx = ib[:, W * KLO * 8:W * KLO * 8 + W * KHI * 8]
            kidxs = ib[:, W * KLO * 8 + W * KHI * 8:NI16]

            CH = 2  # 512-node tiles per xT load chunk
            with tc.tile_pool(name="pa", bufs=2) as pa, \
                 tc.tile_pool(name="pax", bufs=2) as pax, \
                 tc.tile_pool(name="pbc", bufs=2) as pbc, \
                 tc.tile_pool(name="pmtw", bufs=1) as pmtw, \
                 tc.tile_pool(name="pghi", bufs=3) as pghi, \
                 tc.tile_pool(name="pe2", bufs=2) as pe2, \
                 tc.tile_pool(name="pko", bufs=1) as pko, \
                 tc.tile_pool(name="psa", bufs=2, space="PSUM") as psa, \
                 tc.tile_pool(name="psb", bufs=2, space="PSUM") as psb, \
                 tc.tile_pool(name="psc", bufs=1, space="PSUM") as psc, \
                 tc.tile_pool(name="psd", bufs=1, space="PSUM") as psd, \
                 tc.tile_pool(name="pse", bufs=2, space="PSUM") as pse:

                # ---- partition-broadcast of per-slot target cols (Pool) ----
                pbcs = []
                for w in range(W):
                    pbcw = pbc.tile([P, cap], BF, tag="pbcw")
                    nc.gpsimd.partition_broadcast(
                        pbcw[:], etws[0:1, w * cap:(w + 1) * cap])
                    pbcs.append(pbcw)

                # deferred emissions interleaved into phase A slack
                mtws = {}

                def emit_mtw(w):
                    Mtw = pmtw.tile([P, cap], BF, tag="Mtw")
                    nc.vector.tensor_scalar(Mtw[:], pbcs[w][:], iotac[:], None,
                                            OP.is_equal)
                    mtws[w] = Mtw

                def emit_loop1(w):
                    # s_trg / skip for the window targets
                    stp = psd.tile([P, OC], F32, tag="misc")
                    nc.tensor.matmul(stp[:, 0:NH],
                                     lhsT=hfmt[:, w * P:(w + 1) * P],
                                     rhs=btrgs[:], start=True, stop=True)
                    nc.vector.tensor_copy(strg[:, w * NH:(w + 1) * NH],
                                            stp[:, 0:NH])
                    skp = psd.tile([P, OC], F32, tag="misc")
                    nc.tensor.matmul(skp[:], lhsT=hfmt[:, w * P:(w + 1) * P],
                                     rhs=wsks[:], start=True, stop=True)
                    nc.vector.tensor_copy(skips[:, w], skp[:])
                    # s_trg edge-slot expansion via the col-major one-hot
                    Mtw = mtws.pop(w)
                    stps = psc.tile([P, EC, NH], F32, tag="stps")
                    for j in range(EC):
                        nc.tensor.matmul(
                            stps[:, j, :], lhsT=Mtw[:, j * P:(j + 1) * P],
                            rhs=strg[:, w * NH:(w + 1) * NH],
                            start=True, stop=True)
                    nc.vector.tensor_copy(st_sb[:, w], stps[:])

                def emit_mw(w, j):
                    col = w * EC + j
                    nc.vector.tensor_scalar(
                        Mw[:, col * P:(col + 1) * P], iotaL[:, 0:P],
                        etcols[:, col:col + 1], None, OP.is_equal)

                # schedule: loop1(w) at tile 2+w; Mw slots spread over tiles

                # ---------------- phase A ----------------
                for t0 in range(0, NT, CH):
                    wdc = CH * 512
                    xc = pax.tile([P, 2 * wdc], BF, tag="xc")
                    nc.sync.dma_start(xc[:], xTi[:, 2 * t0 * 512:
                                                 2 * t0 * 512 + 2 * wdc])
                    stg = pa.tile([P, 2, 4, 256], BF, tag="stg")
                    for t in range(t0, t0 + CH):
                        o = (t - t0) * 512
                        hps = psa.tile([P, 512], F32, tag="hps")
                        nc.tensor.matmul(hps[:], lhsT=w1a[:],
                                         rhs=xc[:, o:o + 512],
                                         start=True, stop=False)
                        nc.tensor.matmul(hps[:], lhsT=w1b[:],
                                         rhs=xc[:, wdc + o:wdc + o + 512],
                                         start=False, stop=True)
                        hsb = pa.tile([P, 512], BF, tag="hsb")
                        nc.scalar.activation(hsb[:], hps[:], AF.Relu,
                                             bias=b1s[:])
                        if t * 512 < TP:
                            w0 = t * 512
                            w1_ = min(TP, (t + 1) * 512)
                            nc.scalar.activation(hfmt[:, w0:w1_],
                                                 hps[:, 0:(w1_ - w0)], AF.Relu,
                                                 bias=b1s[:])
                        if t == 14:
                            # gate the Mw storm on mid-phase progress:
                            # iotaL = iota4s + 0*hsb  (data dep on tile 14)
                            zt = pa.tile([P, P], BF, tag="zt")
                            nc.vector.tensor_scalar(zt[:], hsb[:, 0:P],
                                                    0.0, None, OP.mult)
                            nc.vector.tensor_add(iotaL[:], iota4s[:, 0:P],
                                                 zt[:])
                        for half in range(2):
                            p2 = psb.tile([P, 2, 256], F32, tag="p2")
                            for jj in range(2):
                                j = half * 2 + jj
                                nc.tensor.matmul(
                                    p2[:, jj, :],
                                    lhsT=hsb[:, j * P:(j + 1) * P],
                                    rhs=w2s[:], start=True, stop=True)
                            sgh = stg[:, t - t0, half * 2:half * 2 + 2, :]
                            if half == 0:
                                nc.scalar.activation(sgh[:, :, 0:OC + NH],
                                                     p2[:, :, 0:OC + NH],
                                                     AF.Copy)
                            else:
                                nc.vector.tensor_copy(sgh[:, :, 0:OC + NH],
                                                      p2[:, :, 0:OC + NH])
                    r0 = t0 * 512
                    tab, rr = (tabLo, r0) if r0 < B1 else (tabHi, r0 - B1)
                    nc.sync.dma_start(
                        tab[rr:rr + CH * 512, :].rearrange(
                            "(i j p) f -> p i j f", p=P, i=CH), stg[:])
                    # interleaved loop-1 / mask emissions
                    for t in range(t0, t0 + CH):
                        if 1 <= t <= 2 * W and t % 2 == 1:
                            emit_mtw((t - 1) // 2)
                        if 2 <= t <= 2 * W + 1 and t % 2 == 0:
                            emit_loop1((t - 2) // 2)

                # edge->target one-hot masks: fills the DVE gap between
                # phase A and the window chains
                for w_ in range(W):
                    for j_ in range(EC):
                        emit_mw(w_, j_)

                # ---------------- gathers ----------------
                # lo gathers read only tabLo: they fire as soon as the lo
                # part of the table is written (overlaps phase A)
                for w in range(W):
                    nc.gpsimd.dma_gather(
                        Glo[:, w], tabLo[:],
                        eloidx[:, w * KLO * 8:(w + 1) * KLO * 8],
                        KLO * P, KLO * P, 256, single_packet=False)
                ghis = []
                for w in range(W):
                    G = pghi.tile([P, KHI, 256], BF, tag="G")
                    nc.gpsimd.dma_gather(
                        G[:], tabHi[:],
                        ehiidx[:, w * KHI * 8:(w + 1) * KHI * 8],
                        KHI * P, KHI * P, 256, single_packet=False)
                    ghis.append(G)

                # ---------------- loop 2: per-window edge pipeline ----------
                def finalize(w, segp):
                    den = pe2.tile([P, NH], F32, tag="den")
                    nc.vector.tensor_scalar_add(den[:], segp[:, OC:OC + NH],
                                                EPS)
                    rec = pe2.tile([P, NH], F32, tag="rec")
                    nc.vector.reciprocal(rec[:], den[:])
                    z = pe2.tile([P, OC], F32, tag="z")
                    recb = rec[:].broadcast_to([P, NH, HD])
                    nc.vector.tensor_tensor(
                        z[:].rearrange("p (a d) -> p a d", d=HD),
                        segp[:, 0:OC].rearrange("p (a d) -> p a d", d=HD),
                        recb, OP.mult)
                    nc.gpsimd.tensor_add(z[:], z[:], skips[:, w])
                    # elu: (max(z,0)-1) + exp(min(z,0))
                    am = pe2.tile([P, OC], BF, tag="am")
                    nc.gpsimd.tensor_scalar(am[:], z[:], 0.0, -1.0, OP.max,
                                            OP.add)
                    bm = pe2.tile([P, OC], BF, tag="bm")
                    nc.gpsimd.tensor_scalar(bm[:], z[:], 0.0, None, OP.min)
                    eb = pe2.tile([P, OC], BF, tag="eb")
                    nc.scalar.activation(eb[:], bm[:], AF.Exp)
                    fo = pe2.tile([P, OC], BF, tag="fo")
                    nc.vector.tensor_add(fo[:], am[:], eb[:])
                    nc.sync.dma_start(outT[w * P:(w + 1) * P, :], fo[:])

                pending = None
                for w in range(W):
                    G = ghis[w]
                    # scores = s_src(gathered) + s_trg(expanded)
                    sc = pe2.tile([P, EC, NH], F32, tag="sc")
                    glo_ss = Glo[:, w, :, OC:OC + NH]
                    nc.vector.tensor_tensor(sc[:, 0:KLO], st_sb[:, w, 0:KLO],
                                            glo_ss, OP.add)
                    if KHI:
                        nc.vector.tensor_tensor(sc[:, KLO:EC],
                                                st_sb[:, w, KLO:EC],
                                                G[:, :, OC:OC + NH], OP.add)
                    # e = exp(leakyrelu(s)) = max(exp(s), exp(0.2 s))
                    e1 = pe2.tile([P, EC, NH], BF, tag="e1")
                    nc.scalar.activation(e1[:], sc[:], AF.Exp)
                    e2 = pe2.tile([P, EC, NH], BF, tag="e2")
                    nc.scalar.activation(e2[:], sc[:], AF.Exp, scale=0.2)
                    emax = pe2.tile([P, EC, NH], BF, tag="emax")
                    nc.vector.tensor_max(emax[:], e1[:], e2[:])
                    Wv = pe2.tile([P, EC, 136], BF, tag="Wv")
                    nc.vector.tensor_copy(Wv[:, :, OC:OC + NH], emax[:])
                    emb = emax[:].broadcast_to([P, EC, NH, HD])
                    wv4 = Wv[:, :, 0:OC].rearrange("p j (a d) -> p j a d", d=HD)
                    if w % 2 == 0:
                        # expand e per-head on ACT, packed bf16 mult on DVE
                        eex = pmtw.tile([P, cap], BF, tag="Mtw")
                        ex3 = eex[:].rearrange("p (j f) -> p j f", f=P)
                        nc.scalar.activation(
                            ex3.rearrange("p j (a d) -> p j a d", d=HD),
                            emb, AF.Copy)
                        nc.vector.tensor_tensor(Wv[:, 0:KLO, 0:OC],
                                                Glo[:, w, :, 0:OC],
                                                ex3[:, 0:KLO], OP.mult)
                        nc.vector.tensor_tensor(Wv[:, KLO:EC, 0:OC],
                                                G[:, :, 0:OC],
                                                ex3[:, KLO:EC], OP.mult)
                    else:
                        # same eex path, rotating through the dead pbc slot
                        eex = pbc.tile([P, cap], BF, tag="pbcw")
                        ex3 = eex[:].rearrange("p (j f) -> p j f", f=P)
                        nc.scalar.activation(
                            ex3.rearrange("p j (a d) -> p j a d", d=HD),
                            emb, AF.Copy)
                        nc.vector.tensor_tensor(Wv[:, 0:KLO, 0:OC],
                                                Glo[:, w, :, 0:OC],
                                                ex3[:, 0:KLO], OP.mult)
                        nc.vector.tensor_tensor(Wv[:, KLO:EC, 0:OC],
                                                G[:, :, 0:OC],
                                                ex3[:, KLO:EC], OP.mult)

                    segp = pse.tile([P, 136], F32, tag="segp")
                    for j in range(EC):
                        nc.tensor.matmul(segp[:],
                                         lhsT=Mw[:, (w * EC + j) * P:
                                                 (w * EC + j + 1) * P],
                                         rhs=Wv[:, j, :], start=(j == 0),
                                         stop=(j == EC - 1))
                    if pending is not None:
                        finalize(*pending)
                    pending = (w, segp)
                if pending is not None:
                    finalize(*pending)

                # final k-row gather
                ko = pko.tile([P, TP // P, OC], BF, tag="ko")
                nc.gpsimd.dma_gather(ko[:], outT[:], kidxs[:], TP, TP, OC,
                                     single_packet=False)
                nc.sync.dma_start(
                    out[:].rearrange("(j p) f -> p j f", p=P), ko[:])

    nc.compile()
    _BUILD_CACHE[key] = nc
    return nc


# ----------------------------------------------------------------------------
# entry point
# ----------------------------------------------------------------------------

def kernel(x, adj0, index0, w_in, b_in, w_proj, a_src, a_trg, w_skip):
    from concourse.bass_utils import run_bass_kernel_spmd

    per_core, NPAD, EC, KLO, B1 = plan(x, adj0, index0)
    wts = make_weights(w_in, b_in, w_proj, a_src, a_trg, w_skip)
    nc = build(NPAD, EC, KLO, B1)

    in_maps = []
    for c in range(CORES):
        pc = per_core[c]
        fblob = np.concatenate(
            [wts["b1"], wts["iota_c"], pc["etcol"]], axis=1).astype(np.float32)
        in_maps.append(dict(bfblob=wts["bfblob"], fblob=fblob,
                            xTi=pc["xTi"], iblob=pc["iblob"],
                            etrow=pc["etrow"]))

    res = run_bass_kernel_spmd(nc, in_maps, core_ids=list(range(CORES)))
    outs = [r["out"][:KC] for r in res.results]
    return np.concatenate(outs, axis=0).astype(np.float32)


# revision 29
# speedup vs baseline: 1.0129x; 1.0129x over previous
"""GAT BasicAttentionBlock kernel for 8x Trainium2 NeuronCores.

Strategy (output-shard, v2): each core owns 1250 of the 10000 selected
output rows (index0).  Only nodes reachable from those rows matter
(~1.2k targets + ~16k sources per core).  Per core:

  node table order: [window-grouped targets (1280 rows) | sources sorted
  by per-core edge multiplicity desc].  A static row boundary B1 (mult
  of 512) splits the table so rows < B1 carry ~2/3 of the edges.

  phase A  stream x columns of the table nodes (bf16), h = relu(x@w1.T)
           feature-major on PE, then per 128-node subtile
           proj|s_src = h@w2 node-major; copy cols 0:136 (bf16) of each
           PSUM tile to SBUF and DMA full 512-byte rows to the HBM table.
           Emission of loop-1 work (s_trg/skip/one-hot masks) is
           interleaved into phase A's engine slack.
  gathers  per 128-target window, edges sorted by source row: slots
           [0,KLO) hold only sources < B1 and are gathered as soon as
           the lo part of the table is written (overlaps phase A);
           slots [KLO,EC) gather after the full table.
  loop 2   per window: scores = lrelu(s_src+s_trg) on ACT (alpha=.2),
           e = exp, weighted = e*proj, segment-sum via one-hot matmuls
           accumulated in PSUM [sum e*proj | sum e]; out = att/den +
           skip, ELU; windows finish staggered as hi-gathers land.
  final    dma_gather the 1250 output rows from the 1280-target table.

No collectives: cores are fully independent.  The softmax global max
subtraction cancels in att = exp/sum(exp) and is dropped.
"""

import os
import sys

for _p in ("/opt/trn_rl_repo",):
    if os.path.isdir(_p) and _p not in sys.path:
        sys.path.insert(0, _p)

import numpy as np
import ml_dtypes

# problem constants (hardcoded per contract)
N = 50000
E = 800000
K = 10000
IN = 256
H = 128
NH = 8
HD = 16
OC = NH * HD  # 128
CORES = 8
KC = K // CORES          # 1250 output rows per core
P = 128
W = 10                   # target windows of 128 -> 1280 target slots
TP = W * P               # padded target count per core
EPS = 1e-16

BF16 = ml_dtypes.bfloat16

LOFRAC = float(os.environ.get("KLOFRAC", "0.55"))


# ----------------------------------------------------------------------------
# host-side sharding / planning
# ----------------------------------------------------------------------------

def _wrap16(vals, reps=8):
    """int16 index layout for dma_gather: idx i at [i%16, i//16], the 16-row
    block replicated `reps` times down the partition axis."""
    L = vals.shape[0]
    assert L % 16 == 0
    w = vals.reshape(L // 16, 16).T.astype(np.int16)
    return np.tile(w, (reps, 1))


def _binpack(deg):
    """Assign targets (by degree desc) to W windows (<=128 each), balancing
    total degree.  Returns row index (w*128 + pos) per target."""
    U = len(deg)
    order = np.argsort(-deg, kind="stable")
    wdeg = np.zeros(W)
    wcnt = np.zeros(W, np.int64)
    row = np.zeros(U, np.int64)
    for u in order:
        cand = np.nonzero(wcnt < P)[0]
        wsel = cand[np.argmin(wdeg[cand])]
        row[u] = wsel * P + wcnt[wsel]
        wcnt[wsel] += 1
        wdeg[wsel] += deg[u]
    return row


def plan(x, adj0, index0):
    src_all = np.asarray(adj0[0], dtype=np.int64)
    trg_all = np.asarray(adj0[1], dtype=np.int64)
    idx0 = np.asarray(index0, dtype=np.int64)
    x = np.asarray(x, dtype=np.float32)

    pre = []
    npad_req = 512
    for c in range(CORES):
        ks = idx0[c * KC:(c + 1) * KC]
        tgt_u, inv_k = np.unique(ks, return_inverse=True)
        U_t = len(tgt_u)
        assert U_t <= TP
        lut = np.full(N, -1, np.int64)
        lut[tgt_u] = np.arange(U_t)
        tloc_all = lut[trg_all]
        sel = np.nonzero(tloc_all >= 0)[0]
        e_src = src_all[sel]
        e_tu = tloc_all[sel]
        deg = np.bincount(e_tu, minlength=U_t)
        trow = _binpack(deg)                       # tgt_u idx -> table row

        # source rows: targets keep their rows; extras sorted by edge count
        nrow = np.full(N, -1, np.int64)
        nrow[tgt_u] = trow
        is_extra = nrow[e_src] < 0
        ex_ids, ex_cnt_inv = np.unique(e_src[is_extra], return_inverse=True)
        ex_cnt = np.bincount(ex_cnt_inv)
        ex_order = np.argsort(-ex_cnt, kind="stable")
        extras = ex_ids[ex_order]
        nrow[extras] = TP + np.arange(len(extras))
        U_n = TP + len(extras)
        npad_req = max(npad_req, U_n)

        e_srow = nrow[e_src]                       # source table row per edge
        e_trow = trow[e_tu]                        # target table row per edge
        # node id per table row (for xT); pad rows -> x of node 0 (harmless)
        nodes = np.zeros(U_n, np.int64)
        nodes[trow] = tgt_u
        nodes[TP:] = extras
        pre.append((trow, inv_k, e_srow, e_trow, nodes, U_n))

    NPAD = ((npad_req + 511) // 512) * 512

    # shared lo-chunk boundary B1 (mult of 512): rows < B1 carry >= LOFRAC
    # of edges on every core
    b1_req = 512
    for c in range(CORES):
        _, _, e_srow, _, _, _ = pre[c]
        hist = np.bincount(e_srow // 512, minlength=NPAD // 512)
        cum = np.cumsum(hist) / len(e_srow)
        t = int(np.searchsorted(cum, LOFRAC)) + 1
        b1_req = max(b1_req, t * 512)
    B1 = min((b1_req + 1023) // 1024 * 1024, NPAD)

    # per-core, per-window edge packing: strict lo/hi slot segregation
    # (lo gathers read only tabLo -> they can fire during phase A)
    klm_req = 1
    khi_req = 1
    packed = []
    for c in range(CORES):
        trow, inv_k, e_srow, e_trow, nodes, U_n = pre[c]
        e_win = e_trow >> 7
        is_lo = e_srow < B1
        order = np.lexsort((~is_lo, e_win))
        e_srow = e_srow[order]
        e_trow = e_trow[order]
        e_win = e_win[order]
        nlo = np.bincount(e_win[e_srow < B1], minlength=W)
        nhi = np.bincount(e_win[e_srow >= B1], minlength=W)
        klm_req = max(klm_req, int(np.ceil(nlo.max() / P)))
        khi_req = max(khi_req, int(np.ceil(nhi.max() / P)))
        packed.append((trow, inv_k, e_srow, e_trow, e_win, nlo, nhi, nodes))

    KLO = klm_req
    KHI = khi_req
    EC = KLO + KHI
    cap = EC * P

    per_core = []
    for c in range(CORES):
        trow, inv_k, e_srow, e_trow, e_win, nlo, nhi, nodes = packed[c]
        # slot within window: lo edges at [0, nlo), hi at [KLO*P, KLO*P+nhi)
        cnt = nlo + nhi
        start = np.concatenate([[0], np.cumsum(cnt)[:-1]])
        within = np.arange(len(e_trow)) - start[e_win]
        is_hi = within >= nlo[e_win]
        within = within + is_hi * (KLO * P - nlo[e_win])
        slots = e_win * cap + within

        esrc_flat = np.zeros(W * cap, np.int64)
        # dummy hi slots point at tabHi row 0
        for w in range(W):
            esrc_flat[w * cap + KLO * P:(w + 1) * cap] = B1
        etcol_flat = np.full(W * cap, -1.0, np.float32)
        esrc_flat[slots] = e_srow
        etcol_flat[slots] = (e_trow - e_win * P).astype(np.float32)

        etcol = etcol_flat.reshape(W, EC, P).transpose(2, 0, 1).reshape(P, W * EC)
        etrow_b = etcol_flat.astype(BF16).reshape(1, W * cap)
        eidx_lo = np.concatenate(
            [_wrap16(esrc_flat[w * cap:w * cap + KLO * P]) for w in range(W)],
            axis=1)
        eidx_hi = np.concatenate(
            [_wrap16(esrc_flat[w * cap + KLO * P:(w + 1) * cap] - B1)
             for w in range(W)], axis=1)

        kvals = np.zeros(TP, np.int64)
        kvals[:KC] = trow[inv_k]
        kidx = _wrap16(kvals)

        xT = np.zeros((IN, NPAD), BF16)
        xT[:, :len(nodes)] = x[nodes].T
        # interleave the two 128-row halves chunk-wise: one DMA per chunk
        CW = 1024  # CH(=2) * 512
        assert NPAD % CW == 0
        xTi = np.empty((P, 2 * NPAD), BF16)
        for c in range(NPAD // CW):
            xTi[:, 2 * c * CW:2 * c * CW + CW] = xT[0:P, c * CW:(c + 1) * CW]
            xTi[:, 2 * c * CW + CW:2 * (c + 1) * CW] = \
                xT[P:IN, c * CW:(c + 1) * CW]

        iblob = np.concatenate([eidx_lo, eidx_hi, kidx], axis=1)
        per_core.append(dict(xTi=xTi, iblob=iblob,
                             etcol=etcol, etrow=etrow_b))
    return per_core, NPAD, EC, KLO, B1


def make_weights(w_in, b_in, w_proj, a_src, a_trg, w_skip):
    w_in = np.asarray(w_in, np.float32)
    b_in = np.asarray(b_in, np.float32)
    w_proj = np.asarray(w_proj, np.float32)
    a_src = np.asarray(a_src, np.float32).reshape(NH, HD)
    a_trg = np.asarray(a_trg, np.float32).reshape(NH, HD)
    w_skip = np.asarray(w_skip, np.float32)

    w1T = np.ascontiguousarray(w_in.T).astype(BF16)        # [256,128]
    b1 = b_in.reshape(H, 1).astype(np.float32)
    # B_src[h, a] = sum_d w_proj[a*16+d, h] * a_src[a, d]
    wp3 = w_proj.reshape(NH, HD, H)
    B_src = np.einsum("adh,ad->ha", wp3, a_src).astype(np.float32)  # [128,8]
    B_trg = np.einsum("adh,ad->ha", wp3, a_trg).astype(BF16)
    w2 = np.zeros((H, 256), np.float32)  # cast to bf16 below
    w2[:, :OC] = w_proj.T
    w2[:, OC:OC + NH] = B_src
    wskT = np.ascontiguousarray(w_skip.T).astype(BF16)     # [128,128]
    iota4 = np.tile(np.arange(P, dtype=BF16)[None, :], (P, 2))
    iota_c = np.arange(P, dtype=np.float32).reshape(P, 1)
    bfblob = np.concatenate(
        [np.ascontiguousarray(w1T[0:P]), np.ascontiguousarray(w1T[P:IN]),
         w2.astype(BF16), wskT, B_trg, iota4], axis=1)  # [128, 776]
    return dict(bfblob=bfblob, b1=b1, iota_c=iota_c)


# ----------------------------------------------------------------------------
# bass kernel
# ----------------------------------------------------------------------------

_BUILD_CACHE = {}


def build(NPAD, EC, KLO, B1):
    key = (NPAD, EC, KLO, B1)
    if key in _BUILD_CACHE:
        return _BUILD_CACHE[key]

    import concourse.bacc as bacc
    import concourse.mybir as mybir
    import concourse.tile as tile

    dt = mybir.dt
    F32 = dt.float32
    F32R = dt.float32r
    I16 = dt.int16
    BF = dt.bfloat16
    AF = mybir.ActivationFunctionType
    OP = mybir.AluOpType

    NT = NPAD // 512
    cap = EC * P
    KHI = EC - KLO

    nc = bacc.Bacc("TRN2", target_bir_lowering=False)

    with tile.TileContext(nc) as tc:
        with tc.tile_pool(name="dram", bufs=1, space="DRAM") as dram:
            def din(name, shape, dtp):
                return dram.tile(shape, dtp, kind="ExternalInput", name=name,
                                 uniquify=False)

            NBF = H + H + 256 + OC + NH + 2 * P  # 904
            NI16 = W * KLO * 8 + W * KHI * 8 + TP // 16
            xTi = din("xTi", [P, 2 * NPAD], BF)
            bfblob = din("bfblob", [P, NBF], BF)
            fblob = din("fblob", [P, 2 + W * EC], F32)
            iblob = din("iblob", [P, NI16], I16)
            etrow = din("etrow", [1, W * cap], BF)

            tabLo = dram.tile([B1, 256], BF, kind="Internal", name="tabLo",
                              uniquify=False)
            tabHi = dram.tile([NPAD - B1, 256], BF, kind="Internal",
                              name="tabHi", uniquify=False)
            outT = dram.tile([TP, OC], BF, kind="Internal", name="outT",
                             uniquify=False)
            out = dram.tile([TP, OC], BF, kind="ExternalOutput", name="out",
                            uniquify=False)

        with tc.tile_pool(name="pers", bufs=1) as pers:
            bfb = pers.tile([P, NBF], BF)
            fb = pers.tile([P, 2 + W * EC], F32)
            ib = pers.tile([P, NI16], I16)
            hfmt = pers.tile([H, TP], BF)         # targets' h, feature-major
            strg = pers.tile([P, W * NH], BF)     # per-window s_trg  [t, 8]
            skips = pers.tile([P, W, OC], BF)     # per-window skip   [t, oc]
            st_sb = pers.tile([P, W, EC, NH], BF)   # s_trg per edge slot
            Mw = pers.tile([P, W * cap], BF)      # edge->target one-hot
            iotaL = pers.tile([P, P], BF)         # iota copy, gated mid-phase
            Glo = pers.tile([P, W, KLO, 256], BF)  # lo-gathered table rows
            etws = pers.tile([1, W * cap], BF)

            nc.sync.dma_start(etws[:], etrow[:])
            nc.sync.dma_start(fb[:], fblob[:])
            nc.sync.dma_start(bfb[:], bfblob[:])
            nc.sync.dma_start(ib[:], iblob[:])

            w1a = bfb[:, 0:H]
            w1b = bfb[:, H:2 * H]
            w2s = bfb[:, 2 * H:2 * H + 256]
            wsks = bfb[:, 2 * H + 256:2 * H + 256 + OC]
            btrgs = bfb[:, 2 * H + 256 + OC:2 * H + 256 + OC + NH]
            iota4s = bfb[:, 2 * H + 256 + OC + NH:NBF]
            b1s = fb[:, 0:1]
            iotac = fb[:, 1:2]
            etcols = fb[:, 2:2 + W * EC]
            eloidx = ib[:, 0:W * KLO * 8]
            ehiidx = ib[:, W * KLO * 8:W * KLO * 8 + W * KHI * 8]
            kidxs = ib[:, W * KLO * 8 + W * KHI * 8:NI16]

            CH = 2  # 512-node tiles per xT load chunk
            with tc.tile_pool(name="pa", bufs=2) as pa, \
                 tc.tile_pool(name="pax", bufs=2) as pax, \
                 tc.tile_pool(name="pbc", bufs=2) as pbc, \
                 tc.tile_pool(name="pmtw", bufs=1) as pmtw, \
                 tc.tile_pool(name="pghi", bufs=3) as pghi, \
                 tc.tile_pool(name="pe2", bufs=2) as pe2, \
                 tc.tile_pool(name="pko", bufs=1) as pko, \
                 tc.tile_pool(name="psa", bufs=2, space="PSUM") as psa, \
                 tc.tile_pool(name="psb", bufs=2, space="PSUM") as psb, \
                 tc.tile_pool(name="psc", bufs=1, space="PSUM") as psc, \
                 tc.tile_pool(name="psd", bufs=1, space="PSUM") as psd, \
                 tc.tile_pool(name="pse", bufs=2, space="PSUM") as pse:

                # ---- partition-broadcast of per-slot target cols (Pool) ----
                pbcs = []
                for w in range(W):
                    pbcw = pbc.tile([P, cap], BF, tag="pbcw")
                    nc.gpsimd.partition_broadcast(
                        pbcw[:], etws[0:1, w * cap:(w + 1) * cap])
                    pbcs.append(pbcw)

                # deferred emissions interleaved into phase A slack
                mtws = {}

                def emit_mtw(w):
                    Mtw = pmtw.tile([P, cap], BF, tag="Mtw")
                    nc.vector.tensor_scalar(Mtw[:], pbcs[w][:], iotac[:], None,
                                            OP.is_equal)
                    mtws[w] = Mtw

                def emit_loop1(w):
                    # s_trg / skip for the window targets
                    stp = psd.tile([P, OC], F32, tag="misc")
                    nc.tensor.matmul(stp[:, 0:NH],
                                     lhsT=hfmt[:, w * P:(w + 1) * P],
                                     rhs=btrgs[:], start=True, stop=True)
                    nc.vector.tensor_copy(strg[:, w * NH:(w + 1) * NH],
                                            stp[:, 0:NH])
                    skp = psd.tile([P, OC], F32, tag="misc")
                    nc.tensor.matmul(skp[:], lhsT=hfmt[:, w * P:(w + 1) * P],
                                     rhs=wsks[:], start=True, stop=True)
                    nc.vector.tensor_copy(skips[:, w], skp[:])
                    # s_trg edge-slot expansion via the col-major one-hot
                    Mtw = mtws.pop(w)
                    stps = psc.tile([P, EC, NH], F32, tag="stps")
                    for j in range(EC):
                        nc.tensor.matmul(
                            stps[:, j, :], lhsT=Mtw[:, j * P:(j + 1) * P],
                            rhs=strg[:, w * NH:(w + 1) * NH],
                            start=True, stop=True)
                    nc.vector.tensor_copy(st_sb[:, w], stps[:])

                def emit_mw(w, j):
                    col = w * EC + j
                    nc.vector.tensor_scalar(
                        Mw[:, col * P:(col + 1) * P], iotaL[:, 0:P],
                        etcols[:, col:col + 1], None, OP.is_equal)

                # schedule: loop1(w) at tile 2+w; Mw slots spread over tiles

                # ---------------- phase A ----------------
                for t0 in range(0, NT, CH):
                    wdc = CH * 512
                    xc = pax.tile([P, 2 * wdc], BF, tag="xc")
                    nc.sync.dma_start(xc[:], xTi[:, 2 * t0 * 512:
                                                 2 * t0 * 512 + 2 * wdc])
                    stg = pa.tile([P, 2, 4, 256], BF, tag="stg")
                    for t in range(t0, t0 + CH):
                        o = (t - t0) * 512
                        hps = psa.tile([P, 512], F32, tag="hps")
                        nc.tensor.matmul(hps[:], lhsT=w1a[:],
                                         rhs=xc[:, o:o + 512],
                                         start=True, stop=False)
                        nc.tensor.matmul(hps[:], lhsT=w1b[:],
                                         rhs=xc[:, wdc + o:wdc + o + 512],
                                         start=False, stop=True)
                        hsb = pa.tile([P, 512], BF, tag="hsb")
                        nc.scalar.activation(hsb[:], hps[:], AF.Relu,
                                             bias=b1s[:])
                        if t * 512 < TP:
                            w0 = t * 512
                            w1_ = min(TP, (t + 1) * 512)
                            nc.scalar.activation(hfmt[:, w0:w1_],
                                                 hps[:, 0:(w1_ - w0)], AF.Relu,
                                                 bias=b1s[:])
                        if t == 14:
                            # gate the Mw storm on mid-phase progress:
                            # iotaL = iota4s + 0*hsb  (data dep on tile 14)
                            zt = pa.tile([P, P], BF, tag="zt")
                            nc.vector.tensor_scalar(zt[:], hsb[:, 0:P],
                                                    0.0, None, OP.mult)
                            nc.vector.tensor_add(iotaL[:], iota4s[:, 0:P],
                                                 zt[:])
                        for half in range(2):
                            p2 = psb.tile([P, 2, 256], F32, tag="p2")
                            for jj in range(2):
                                j = half * 2 + jj
                                nc.tensor.matmul(
                                    p2[:, jj, :],
                                    lhsT=hsb[:, j * P:(j + 1) * P],
                                    rhs=w2s[:], start=True, stop=True)
                            sgh = stg[:, t - t0, half * 2:half * 2 + 2, :]
                            if half == 0:
                                nc.scalar.activation(sgh[:, :, 0:OC + NH],
                                                     p2[:, :, 0:OC + NH],
                                                     AF.Copy)
                            else:
                                nc.vector.tensor_copy(sgh[:, :, 0:OC + NH],
                                                      p2[:, :, 0:OC + NH])
                    r0 = t0 * 512
                    tab, rr = (tabLo, r0) if r0 < B1 else (tabHi, r0 - B1)
                    nc.sync.dma_start(
                        tab[rr:rr + CH * 512, :].rearrange(
                            "(i j p) f -> p i j f", p=P, i=CH), stg[:])
                    # interleaved loop-1 / mask emissions
                    for t in range(t0, t0 + CH):
                        if 1 <= t <= 2 * W and t % 2 == 1:
                            emit_mtw((t - 1) // 2)
                        if 2 <= t <= 2 * W + 1 and t % 2 == 0:
                            emit_loop1((t - 2) // 2)

                # edge->target one-hot masks: fills the DVE gap between
                # phase A and the window chains
                for w_ in range(W):
                    for j_ in range(EC):
                        emit_mw(w_, j_)

                # ---------------- gathers ----------------
                # lo gathers read only tabLo: they fire as soon as the lo
                # part of the table is written (overlaps phase A)
                for w in range(W):
                    nc.gpsimd.dma_gather(
                        Glo[:, w], tabLo[:],
                        eloidx[:, w * KLO * 8:(w + 1) * KLO * 8],
                        KLO * P, KLO * P, 256, single_packet=False)
                ghis = []
                for w in range(W):
                    G = pghi.tile([P, KHI, 256], BF, tag="G")
                    nc.gpsimd.dma_gather(
                        G[:], tabHi[:],
                        ehiidx[:, w * KHI * 8:(w + 1) * KHI * 8],
                        KHI * P, KHI * P, 256, single_packet=False)
                    ghis.append(G)

                # ---------------- loop 2: per-window edge pipeline ----------
                def finalize(w, segp):
                    den = pe2.tile([P, NH], F32, tag="den")
                    nc.vector.tensor_scalar_add(den[:], segp[:, OC:OC + NH],
                                                EPS)
                    rec = pe2.tile([P, NH], F32, tag="rec")
                    nc.vector.reciprocal(rec[:], den[:])
                    z = pe2.tile([P, OC], F32, tag="z")
                    recb = rec[:].broadcast_to([P, NH, HD])
                    nc.vector.tensor_tensor(
                        z[:].rearrange("p (a d) -> p a d", d=HD),
                        segp[:, 0:OC].rearrange("p (a d) -> p a d", d=HD),
                        recb, OP.mult)
                    nc.gpsimd.tensor_add(z[:], z[:], skips[:, w])
                    # elu: (max(z,0)-1) + exp(min(z,0))
                    am = pe2.tile([P, OC], BF, tag="am")
                    nc.gpsimd.tensor_scalar(am[:], z[:], 0.0, -1.0, OP.max,
                                            OP.add)
                    bm = pe2.tile([P, OC], BF, tag="bm")
                    nc.gpsimd.tensor_scalar(bm[:], z[:], 0.0, None, OP.min)
                    eb = pe2.tile([P, OC], BF, tag="eb")
                    nc.scalar.activation(eb[:], bm[:], AF.Exp)
                    fo = pe2.tile([P, OC], BF, tag="fo")
                    nc.vector.tensor_add(fo[:], am[:], eb[:])
                    nc.sync.dma_start(outT[w * P:(w + 1) * P, :], fo[:])

                # software-pipelined window stages: each engine's
                # in-order queue interleaves adjacent windows
                st1 = {}   # w -> (sc-dependent) emax tile
                st2 = {}   # w -> Wv tile
                st3 = {}   # w -> segp psum tile

                def stage1(w):
                    G = ghis[w]
                    sc = pe2.tile([P, EC, NH], F32, tag="sc")
                    glo_ss = Glo[:, w, :, OC:OC + NH]
                    nc.vector.tensor_tensor(sc[:, 0:KLO], st_sb[:, w, 0:KLO],
                                            glo_ss, OP.add)
                    nc.vector.tensor_tensor(sc[:, KLO:EC],
                                            st_sb[:, w, KLO:EC],
                                            G[:, :, OC:OC + NH], OP.add)
                    e1 = pe2.tile([P, EC, NH], BF, tag="e1")
                    nc.scalar.activation(e1[:], sc[:], AF.Exp)
                    e2 = pe2.tile([P, EC, NH], BF, tag="e2")
                    nc.scalar.activation(e2[:], sc[:], AF.Exp, scale=0.2)
                    emax = pe2.tile([P, EC, NH], BF, tag="emax")
                    nc.vector.tensor_max(emax[:], e1[:], e2[:])
                    st1[w] = emax

                def stage2(w):
                    G = ghis[w]
                    emax = st1.pop(w)
                    Wv = pe2.tile([P, EC, 136], BF, tag="Wv")
                    nc.vector.tensor_copy(Wv[:, :, OC:OC + NH], emax[:])
                    emb = emax[:].broadcast_to([P, EC, NH, HD])
                    pool = pmtw if w % 2 == 0 else pbc
                    tag = "Mtw" if w % 2 == 0 else "pbcw"
                    eex = pool.tile([P, cap], BF, tag=tag)
                    ex3 = eex[:].rearrange("p (j f) -> p j f", f=P)
                    nc.scalar.activation(
                        ex3.rearrange("p j (a d) -> p j a d", d=HD),
                        emb, AF.Copy)
                    nc.vector.tensor_tensor(Wv[:, 0:KLO, 0:OC],
                                            Glo[:, w, :, 0:OC],
                                            ex3[:, 0:KLO], OP.mult)
                    nc.vector.tensor_tensor(Wv[:, KLO:EC, 0:OC],
                                            G[:, :, 0:OC],
                                            ex3[:, KLO:EC], OP.mult)
                    st2[w] = Wv

                def stage3(w):
                    Wv = st2.pop(w)
                    segp = pse.tile([P, 136], F32, tag="segp")
                    for j in range(EC):
                        nc.tensor.matmul(segp[:],
                                         lhsT=Mw[:, (w * EC + j) * P:
                                                 (w * EC + j + 1) * P],
                                         rhs=Wv[:, j, :], start=(j == 0),
                                         stop=(j == EC - 1))
                    st3[w] = segp

                def finalize(w):
                    segp = st3.pop(w)
                    den = pe2.tile([P, NH], F32, tag="den")
                    nc.vector.tensor_scalar_add(den[:], segp[:, OC:OC + NH],
                                                EPS)
                    rec = pe2.tile([P, NH], F32, tag="rec")
                    nc.vector.reciprocal(rec[:], den[:])
                    z = pe2.tile([P, OC], F32, tag="z")
                    recb = rec[:].broadcast_to([P, NH, HD])
                    nc.vector.tensor_tensor(
                        z[:].rearrange("p (a d) -> p a d", d=HD),
                        segp[:, 0:OC].rearrange("p (a d) -> p a d", d=HD),
                        recb, OP.mult)
                    nc.gpsimd.tensor_add(z[:], z[:], skips[:, w])
                    # elu: (max(z,0)-1) + exp(min(z,0))
                    am = pe2.tile([P, OC], BF, tag="am")
                    nc.gpsimd.tensor_scalar(am[:], z[:], 0.0, -1.0, OP.max,
                                            OP.add)
                    bm = pe2.tile([P, OC], BF, tag="bm")
                    nc.gpsimd.tensor_scalar(bm[:], z[:], 0.0, None, OP.min)
                    eb = pe2.tile([P, OC], BF, tag="eb")
                    nc.scalar.activation(eb[:], bm[:], AF.Exp)
                    fo = pe2.tile([P, OC], BF, tag="fo")
                    nc.vector.tensor_add(fo[:], am[:], eb[:])
                    nc.sync.dma_start(outT[w * P:(w + 1) * P, :], fo[:])

                for w in range(W + 3):
                    if w < W:
                        stage1(w)
                    if 1 <= w <= W:
                        stage2(w - 1)
                    if 2 <= w <= W + 1:
                        stage3(w - 2)
                    if 3 <= w:
                        finalize(w - 3)

                # final k-row gather
                ko = pko.tile([P, TP // P, OC], BF, tag="ko")
                nc.gpsimd.dma_gather(ko[:], outT[:], kidxs[:], TP, TP, OC,
                                     single_packet=False)
                nc.sync.dma_start(
                    out[:].rearrange("(j p) f -> p j f", p=P), ko[:])

    nc.compile()
    _BUILD_CACHE[key] = nc
    return nc


# ----------------------------------------------------------------------------
# entry point
# ----------------------------------------------------------------------------

def kernel(x, adj0, index0, w_in, b_in, w_proj, a_src, a_trg, w_skip):
    from concourse.bass_utils import run_bass_kernel_spmd

    per_core, NPAD, EC, KLO, B1 = plan(x, adj0, index0)
    wts = make_weights(w_in, b_in, w_proj, a_src, a_trg, w_skip)
    nc = build(NPAD, EC, KLO, B1)

    in_maps = []
    for c in range(CORES):
        pc = per_core[c]
        fblob = np.concatenate(
            [wts["b1"], wts["iota_c"], pc["etcol"]], axis=1).astype(np.float32)
        in_maps.append(dict(bfblob=wts["bfblob"], fblob=fblob,
                            xTi=pc["xTi"], iblob=pc["iblob"],
                            etrow=pc["etrow"]))

    res = run_bass_kernel_spmd(nc, in_maps, core_ids=list(range(CORES)))
    outs = [r["out"][:KC] for r in res.results]
    return np.concatenate(outs, axis=0).astype(np.float32)


# revision 31
# speedup vs baseline: 1.0687x; 1.0552x over previous
"""GAT BasicAttentionBlock kernel for 8x Trainium2 NeuronCores.

Strategy (output-shard, v2): each core owns 1250 of the 10000 selected
output rows (index0).  Only nodes reachable from those rows matter
(~1.2k targets + ~16k sources per core).  Per core:

  node table order: [window-grouped targets (1280 rows) | sources sorted
  by per-core edge multiplicity desc].  A static row boundary B1 (mult
  of 512) splits the table so rows < B1 carry ~2/3 of the edges.

  phase A  stream x columns of the table nodes (bf16), h = relu(x@w1.T)
           feature-major on PE, then per 128-node subtile
           proj|s_src = h@w2 node-major; copy cols 0:136 (bf16) of each
           PSUM tile to SBUF and DMA full 512-byte rows to the HBM table.
           Emission of loop-1 work (s_trg/skip/one-hot masks) is
           interleaved into phase A's engine slack.
  gathers  per 128-target window, edges sorted by source row: slots
           [0,KLO) hold only sources < B1 and are gathered as soon as
           the lo part of the table is written (overlaps phase A);
           slots [KLO,EC) gather after the full table.
  loop 2   per window: scores = lrelu(s_src+s_trg) on ACT (alpha=.2),
           e = exp, weighted = e*proj, segment-sum via one-hot matmuls
           accumulated in PSUM [sum e*proj | sum e]; out = att/den +
           skip, ELU; windows finish staggered as hi-gathers land.
  final    dma_gather the 1250 output rows from the 1280-target table.

No collectives: cores are fully independent.  The softmax global max
subtraction cancels in att = exp/sum(exp) and is dropped.
"""

import os
import sys

for _p in ("/opt/trn_rl_repo",):
    if os.path.isdir(_p) and _p not in sys.path:
        sys.path.insert(0, _p)

import numpy as np
import ml_dtypes

# problem constants (hardcoded per contract)
N = 50000
E = 800000
K = 10000
IN = 256
H = 128
NH = 8
HD = 16
OC = NH * HD  # 128
CORES = 8
KC = K // CORES          # 1250 output rows per core
P = 128
W = 10                   # target windows of 128 -> 1280 target slots
TP = W * P               # padded target count per core
EPS = 1e-16

BF16 = ml_dtypes.bfloat16

LOFRAC = float(os.environ.get("KLOFRAC", "0.55"))


# ----------------------------------------------------------------------------
# host-side sharding / planning
# ----------------------------------------------------------------------------

def _wrap16(vals, reps=8):
    """int16 index layout for dma_gather: idx i at [i%16, i//16], the 16-row
    block replicated `reps` times down the partition axis."""
    L = vals.shape[0]
    assert L % 16 == 0
    w = vals.reshape(L // 16, 16).T.astype(np.int16)
    return np.tile(w, (reps, 1))


def _binpack(deg):
    """Assign targets (by degree desc) to W windows (<=128 each), balancing
    total degree.  Returns row index (w*128 + pos) per target."""
    U = len(deg)
    order = np.argsort(-deg, kind="stable")
    wdeg = np.zeros(W)
    wcnt = np.zeros(W, np.int64)
    row = np.zeros(U, np.int64)
    for u in order:
        cand = np.nonzero(wcnt < P)[0]
        wsel = cand[np.argmin(wdeg[cand])]
        row[u] = wsel * P + wcnt[wsel]
        wcnt[wsel] += 1
        wdeg[wsel] += deg[u]
    return row


def plan(x, adj0, index0):
    src_all = np.asarray(adj0[0], dtype=np.int64)
    trg_all = np.asarray(adj0[1], dtype=np.int64)
    idx0 = np.asarray(index0, dtype=np.int64)
    x = np.asarray(x, dtype=np.float32)

    pre = []
    npad_req = 512
    for c in range(CORES):
        ks = idx0[c * KC:(c + 1) * KC]
        tgt_u, inv_k = np.unique(ks, return_inverse=True)
        U_t = len(tgt_u)
        assert U_t <= TP
        lut = np.full(N, -1, np.int64)
        lut[tgt_u] = np.arange(U_t)
        tloc_all = lut[trg_all]
        sel = np.nonzero(tloc_all >= 0)[0]
        e_src = src_all[sel]
        e_tu = tloc_all[sel]
        deg = np.bincount(e_tu, minlength=U_t)
        trow = _binpack(deg)                       # tgt_u idx -> table row

        # source rows: targets keep their rows; extras sorted by edge count
        nrow = np.full(N, -1, np.int64)
        nrow[tgt_u] = trow
        is_extra = nrow[e_src] < 0
        ex_ids, ex_cnt_inv = np.unique(e_src[is_extra], return_inverse=True)
        ex_cnt = np.bincount(ex_cnt_inv)
        ex_order = np.argsort(-ex_cnt, kind="stable")
        extras = ex_ids[ex_order]
        nrow[extras] = TP + np.arange(len(extras))
        U_n = TP + len(extras)
        npad_req = max(npad_req, U_n)

        e_srow = nrow[e_src]                       # source table row per edge
        e_trow = trow[e_tu]                        # target table row per edge
        # node id per table row (for xT); pad rows -> x of node 0 (harmless)
        nodes = np.zeros(U_n, np.int64)
        nodes[trow] = tgt_u
        nodes[TP:] = extras
        pre.append((trow, inv_k, e_srow, e_trow, nodes, U_n))

    NPAD = ((npad_req + 511) // 512) * 512

    # shared lo-chunk boundary B1 (mult of 512): rows < B1 carry >= LOFRAC
    # of edges on every core
    b1_req = 512
    for c in range(CORES):
        _, _, e_srow, _, _, _ = pre[c]
        hist = np.bincount(e_srow // 512, minlength=NPAD // 512)
        cum = np.cumsum(hist) / len(e_srow)
        t = int(np.searchsorted(cum, LOFRAC)) + 1
        b1_req = max(b1_req, t * 512)
    B1 = min((b1_req + 1023) // 1024 * 1024, NPAD)

    # per-core, per-window edge packing: strict lo/hi slot segregation
    # (lo gathers read only tabLo -> they can fire during phase A)
    klm_req = 1
    khi_req = 1
    packed = []
    for c in range(CORES):
        trow, inv_k, e_srow, e_trow, nodes, U_n = pre[c]
        e_win = e_trow >> 7
        is_lo = e_srow < B1
        order = np.lexsort((~is_lo, e_win))
        e_srow = e_srow[order]
        e_trow = e_trow[order]
        e_win = e_win[order]
        nlo = np.bincount(e_win[e_srow < B1], minlength=W)
        nhi = np.bincount(e_win[e_srow >= B1], minlength=W)
        klm_req = max(klm_req, int(np.ceil(nlo.max() / P)))
        khi_req = max(khi_req, int(np.ceil(nhi.max() / P)))
        packed.append((trow, inv_k, e_srow, e_trow, e_win, nlo, nhi, nodes))

    KLO = klm_req
    KHI = khi_req
    EC = KLO + KHI
    cap = EC * P

    per_core = []
    for c in range(CORES):
        trow, inv_k, e_srow, e_trow, e_win, nlo, nhi, nodes = packed[c]
        # slot within window: lo edges at [0, nlo), hi at [KLO*P, KLO*P+nhi)
        cnt = nlo + nhi
        start = np.concatenate([[0], np.cumsum(cnt)[:-1]])
        within = np.arange(len(e_trow)) - start[e_win]
        is_hi = within >= nlo[e_win]
        within = within + is_hi * (KLO * P - nlo[e_win])
        slots = e_win * cap + within

        esrc_flat = np.zeros(W * cap, np.int64)
        # dummy hi slots point at tabHi row 0
        for w in range(W):
            esrc_flat[w * cap + KLO * P:(w + 1) * cap] = B1
        etcol_flat = np.full(W * cap, -1.0, np.float32)
        esrc_flat[slots] = e_srow
        etcol_flat[slots] = (e_trow - e_win * P).astype(np.float32)

        etcol = etcol_flat.reshape(W, EC, P).transpose(2, 0, 1).reshape(P, W * EC)
        etrow_b = etcol_flat.astype(BF16).reshape(1, W * cap)
        eidx_lo = np.concatenate(
            [_wrap16(esrc_flat[w * cap:w * cap + KLO * P]) for w in range(W)],
            axis=1)
        eidx_hi = np.concatenate(
            [_wrap16(esrc_flat[w * cap + KLO * P:(w + 1) * cap] - B1)
             for w in range(W)], axis=1)

        kvals = np.zeros(TP, np.int64)
        kvals[:KC] = trow[inv_k]
        kidx = _wrap16(kvals)

        xT = np.zeros((IN, NPAD), BF16)
        xT[:, :len(nodes)] = x[nodes].T
        # interleave the two 128-row halves chunk-wise (one DMA per chunk),
        # packed in PROCESSING order: tabHi chunks first, then tabLo
        CW = 1024  # CH(=2) * 512
        assert NPAD % CW == 0 and B1 % CW == 0
        xTi = np.empty((P, 2 * NPAD), BF16)
        chunk_order = list(range(B1 // CW, NPAD // CW)) + \
            list(range(0, B1 // CW))
        for i, c in enumerate(chunk_order):
            xTi[:, 2 * i * CW:2 * i * CW + CW] = xT[0:P, c * CW:(c + 1) * CW]
            xTi[:, 2 * i * CW + CW:2 * (i + 1) * CW] = \
                xT[P:IN, c * CW:(c + 1) * CW]

        iblob = np.concatenate([eidx_lo, eidx_hi, kidx], axis=1)
        per_core.append(dict(xTi=xTi, iblob=iblob,
                             etcol=etcol, etrow=etrow_b))
    return per_core, NPAD, EC, KLO, B1


def make_weights(w_in, b_in, w_proj, a_src, a_trg, w_skip):
    w_in = np.asarray(w_in, np.float32)
    b_in = np.asarray(b_in, np.float32)
    w_proj = np.asarray(w_proj, np.float32)
    a_src = np.asarray(a_src, np.float32).reshape(NH, HD)
    a_trg = np.asarray(a_trg, np.float32).reshape(NH, HD)
    w_skip = np.asarray(w_skip, np.float32)

    w1T = np.ascontiguousarray(w_in.T).astype(BF16)        # [256,128]
    b1 = b_in.reshape(H, 1).astype(np.float32)
    # B_src[h, a] = sum_d w_proj[a*16+d, h] * a_src[a, d]
    wp3 = w_proj.reshape(NH, HD, H)
    B_src = np.einsum("adh,ad->ha", wp3, a_src).astype(np.float32)  # [128,8]
    B_trg = np.einsum("adh,ad->ha", wp3, a_trg).astype(BF16)
    w2 = np.zeros((H, 256), np.float32)  # cast to bf16 below
    w2[:, :OC] = w_proj.T
    w2[:, OC:OC + NH] = B_src
    wskT = np.ascontiguousarray(w_skip.T).astype(BF16)     # [128,128]
    iota4 = np.tile(np.arange(P, dtype=BF16)[None, :], (P, 2))
    iota_c = np.arange(P, dtype=np.float32).reshape(P, 1)
    bfblob = np.concatenate(
        [np.ascontiguousarray(w1T[0:P]), np.ascontiguousarray(w1T[P:IN]),
         w2.astype(BF16), wskT, B_trg, iota4], axis=1)  # [128, 776]
    return dict(bfblob=bfblob, b1=b1, iota_c=iota_c)


# ----------------------------------------------------------------------------
# bass kernel
# ----------------------------------------------------------------------------

_BUILD_CACHE = {}


def build(NPAD, EC, KLO, B1):
    key = (NPAD, EC, KLO, B1)
    if key in _BUILD_CACHE:
        return _BUILD_CACHE[key]

    import concourse.bacc as bacc
    import concourse.mybir as mybir
    import concourse.tile as tile

    dt = mybir.dt
    F32 = dt.float32
    F32R = dt.float32r
    I16 = dt.int16
    BF = dt.bfloat16
    AF = mybir.ActivationFunctionType
    OP = mybir.AluOpType

    NT = NPAD // 512
    cap = EC * P
    KHI = EC - KLO

    nc = bacc.Bacc("TRN2", target_bir_lowering=False)

    with tile.TileContext(nc) as tc:
        with tc.tile_pool(name="dram", bufs=1, space="DRAM") as dram:
            def din(name, shape, dtp):
                return dram.tile(shape, dtp, kind="ExternalInput", name=name,
                                 uniquify=False)

            NBF = H + H + 256 + OC + NH + 2 * P  # 904
            NI16 = W * KLO * 8 + W * KHI * 8 + TP // 16
            xTi = din("xTi", [P, 2 * NPAD], BF)
            bfblob = din("bfblob", [P, NBF], BF)
            fblob = din("fblob", [P, 2 + W * EC], F32)
            iblob = din("iblob", [P, NI16], I16)
            etrow = din("etrow", [1, W * cap], BF)

            tabLo = dram.tile([B1, 256], BF, kind="Internal", name="tabLo",
                              uniquify=False)
            tabHi = dram.tile([NPAD - B1, 256], BF, kind="Internal",
                              name="tabHi", uniquify=False)
            outT = dram.tile([TP, OC], BF, kind="Internal", name="outT",
                             uniquify=False)
            out = dram.tile([TP, OC], BF, kind="ExternalOutput", name="out",
                            uniquify=False)

        with tc.tile_pool(name="pers", bufs=1) as pers:
            bfb = pers.tile([P, NBF], BF)
            fb = pers.tile([P, 2 + W * EC], F32)
            ib = pers.tile([P, NI16], I16)
            hfmt = pers.tile([H, TP], BF)         # targets' h, feature-major
            strg = pers.tile([P, W * NH], BF)     # per-window s_trg  [t, 8]
            skips = pers.tile([P, W, OC], BF)     # per-window skip   [t, oc]
            st_sb = pers.tile([P, W, EC, NH], BF)   # s_trg per edge slot
            Mw = pers.tile([P, W * cap], BF)      # edge->target one-hot
            iotaL = pers.tile([P, P], BF)         # iota copy, gated mid-phase
            Ghi = pers.tile([P, W, EC - KLO, 256], BF)  # hi-gathered rows
            etws = pers.tile([1, W * cap], BF)

            nc.sync.dma_start(etws[:], etrow[:])
            nc.sync.dma_start(fb[:], fblob[:])
            nc.sync.dma_start(bfb[:], bfblob[:])
            nc.sync.dma_start(ib[:], iblob[:])

            w1a = bfb[:, 0:H]
            w1b = bfb[:, H:2 * H]
            w2s = bfb[:, 2 * H:2 * H + 256]
            wsks = bfb[:, 2 * H + 256:2 * H + 256 + OC]
            btrgs = bfb[:, 2 * H + 256 + OC:2 * H + 256 + OC + NH]
            iota4s = bfb[:, 2 * H + 256 + OC + NH:NBF]
            b1s = fb[:, 0:1]
            iotac = fb[:, 1:2]
            etcols = fb[:, 2:2 + W * EC]
            eloidx = ib[:, 0:W * KLO * 8]
            ehiidx = ib[:, W * KLO * 8:W * KLO * 8 + W * KHI * 8]
            kidxs = ib[:, W * KLO * 8 + W * KHI * 8:NI16]

            CH = 2  # 512-node tiles per xT load chunk
            with tc.tile_pool(name="pa", bufs=2) as pa, \
                 tc.tile_pool(name="pax", bufs=2) as pax, \
                 tc.tile_pool(name="pbc", bufs=2) as pbc, \
                 tc.tile_pool(name="pmtw", bufs=1) as pmtw, \
                 tc.tile_pool(name="pghi", bufs=3) as pghi, \
                 tc.tile_pool(name="pe2", bufs=2) as pe2, \
                 tc.tile_pool(name="pko", bufs=1) as pko, \
                 tc.tile_pool(name="psa", bufs=2, space="PSUM") as psa, \
                 tc.tile_pool(name="psb", bufs=2, space="PSUM") as psb, \
                 tc.tile_pool(name="psc", bufs=1, space="PSUM") as psc, \
                 tc.tile_pool(name="psd", bufs=1, space="PSUM") as psd, \
                 tc.tile_pool(name="pse", bufs=2, space="PSUM") as pse:

                # ---- partition-broadcast of per-slot target cols (Pool) ----
                pbcs = []
                for w in range(W):
                    pbcw = pbc.tile([P, cap], BF, tag="pbcw")
                    nc.gpsimd.partition_broadcast(
                        pbcw[:], etws[0:1, w * cap:(w + 1) * cap])
                    pbcs.append(pbcw)

                # deferred emissions interleaved into phase A slack
                mtws = {}

                def emit_mtw(w):
                    Mtw = pmtw.tile([P, cap], BF, tag="Mtw")
                    nc.vector.tensor_scalar(Mtw[:], pbcs[w][:], iotac[:], None,
                                            OP.is_equal)
                    mtws[w] = Mtw

                def emit_loop1(w):
                    # s_trg / skip for the window targets
                    stp = psd.tile([P, OC], F32, tag="misc")
                    nc.tensor.matmul(stp[:, 0:NH],
                                     lhsT=hfmt[:, w * P:(w + 1) * P],
                                     rhs=btrgs[:], start=True, stop=True)
                    nc.vector.tensor_copy(strg[:, w * NH:(w + 1) * NH],
                                            stp[:, 0:NH])
                    skp = psd.tile([P, OC], F32, tag="misc")
                    nc.tensor.matmul(skp[:], lhsT=hfmt[:, w * P:(w + 1) * P],
                                     rhs=wsks[:], start=True, stop=True)
                    nc.vector.tensor_copy(skips[:, w], skp[:])
                    # s_trg edge-slot expansion via the col-major one-hot
                    Mtw = mtws.pop(w)
                    stps = psc.tile([P, EC, NH], F32, tag="stps")
                    for j in range(EC):
                        nc.tensor.matmul(
                            stps[:, j, :], lhsT=Mtw[:, j * P:(j + 1) * P],
                            rhs=strg[:, w * NH:(w + 1) * NH],
                            start=True, stop=True)
                    nc.vector.tensor_copy(st_sb[:, w], stps[:])

                def emit_mw(w, j):
                    col = w * EC + j
                    nc.vector.tensor_scalar(
                        Mw[:, col * P:(col + 1) * P], iotaL[:, 0:P],
                        etcols[:, col:col + 1], None, OP.is_equal)

                # schedule: loop1(w) at tile 2+w; Mw slots spread over tiles

                # ---------------- phase A (tabHi tiles first) -----------
                torder = list(range(B1 // 512, NT)) + list(range(0, B1 // 512))
                for ci in range(NT // CH):
                    t0 = torder[ci * CH]
                    assert torder[ci * CH + 1] == t0 + 1
                    wdc = CH * 512
                    xc = pax.tile([P, 2 * wdc], BF, tag="xc")
                    nc.sync.dma_start(xc[:], xTi[:, 2 * ci * wdc:
                                                 2 * (ci + 1) * wdc])
                    stg = pa.tile([P, 2, 4, 256], BF, tag="stg")
                    for t in range(t0, t0 + CH):
                        o = (t - t0) * 512
                        hps = psa.tile([P, 512], F32, tag="hps")
                        nc.tensor.matmul(hps[:], lhsT=w1a[:],
                                         rhs=xc[:, o:o + 512],
                                         start=True, stop=False)
                        nc.tensor.matmul(hps[:], lhsT=w1b[:],
                                         rhs=xc[:, wdc + o:wdc + o + 512],
                                         start=False, stop=True)
                        hsb = pa.tile([P, 512], BF, tag="hsb")
                        nc.scalar.activation(hsb[:], hps[:], AF.Relu,
                                             bias=b1s[:])
                        if t * 512 < TP:
                            w0 = t * 512
                            w1_ = min(TP, (t + 1) * 512)
                            nc.scalar.activation(hfmt[:, w0:w1_],
                                                 hps[:, 0:(w1_ - w0)], AF.Relu,
                                                 bias=b1s[:])
                        if ci * CH + (t - t0) == 14:
                            # gate the Mw storm on mid-phase progress:
                            # iotaL = iota4s + 0*hsb  (data dep mid-phase)
                            zt = pa.tile([P, P], BF, tag="zt")
                            nc.vector.tensor_scalar(zt[:], hsb[:, 0:P],
                                                    0.0, None, OP.mult)
                            nc.vector.tensor_add(iotaL[:], iota4s[:, 0:P],
                                                 zt[:])
                        for half in range(2):
                            p2 = psb.tile([P, 2, 256], F32, tag="p2")
                            for jj in range(2):
                                j = half * 2 + jj
                                nc.tensor.matmul(
                                    p2[:, jj, :],
                                    lhsT=hsb[:, j * P:(j + 1) * P],
                                    rhs=w2s[:], start=True, stop=True)
                            sgh = stg[:, t - t0, half * 2:half * 2 + 2, :]
                            if half == 0:
                                nc.scalar.activation(sgh[:, :, 0:OC + NH],
                                                     p2[:, :, 0:OC + NH],
                                                     AF.Copy)
                            else:
                                nc.vector.tensor_copy(sgh[:, :, 0:OC + NH],
                                                      p2[:, :, 0:OC + NH])
                    r0 = t0 * 512
                    tab, rr = (tabLo, r0) if r0 < B1 else (tabHi, r0 - B1)
                    nc.sync.dma_start(
                        tab[rr:rr + CH * 512, :].rearrange(
                            "(i j p) f -> p i j f", p=P, i=CH), stg[:])
                    # interleaved loop-1 / mask emissions (by position)
                    for pi in (ci * CH, ci * CH + 1):
                        if 1 <= pi <= 2 * W and pi % 2 == 1:
                            emit_mtw((pi - 1) // 2)
                        hfp = (NPAD - B1) // 512  # first tabLo position
                        if hfp + 1 <= pi <= hfp + W:
                            emit_loop1(pi - hfp - 1)

                # edge->target one-hot masks: fills the DVE gap between
                # phase A and the window chains
                for w_ in range(W):
                    for j_ in range(EC):
                        emit_mw(w_, j_)

                # ---------------- gathers ----------------
                # tabHi is written first: hi gathers overlap phase A into
                # the persistent Ghi; lo gathers rotate right after tabLo
                for w in range(W):
                    nc.gpsimd.dma_gather(
                        Ghi[:, w], tabHi[:],
                        ehiidx[:, w * KHI * 8:(w + 1) * KHI * 8],
                        KHI * P, KHI * P, 256, single_packet=False)
                glos = []
                for w in range(W):
                    G = pghi.tile([P, KLO, 256], BF, tag="G")
                    nc.gpsimd.dma_gather(
                        G[:], tabLo[:],
                        eloidx[:, w * KLO * 8:(w + 1) * KLO * 8],
                        KLO * P, KLO * P, 256, single_packet=False)
                    glos.append(G)

                # ---------------- loop 2: per-window edge pipeline ----------
                def finalize(w, segp):
                    den = pe2.tile([P, NH], F32, tag="den")
                    nc.vector.tensor_scalar_add(den[:], segp[:, OC:OC + NH],
                                                EPS)
                    rec = pe2.tile([P, NH], F32, tag="rec")
                    nc.vector.reciprocal(rec[:], den[:])
                    z = pe2.tile([P, OC], F32, tag="z")
                    recb = rec[:].broadcast_to([P, NH, HD])
                    nc.vector.tensor_tensor(
                        z[:].rearrange("p (a d) -> p a d", d=HD),
                        segp[:, 0:OC].rearrange("p (a d) -> p a d", d=HD),
                        recb, OP.mult)
                    nc.gpsimd.tensor_add(z[:], z[:], skips[:, w])
                    # elu: (max(z,0)-1) + exp(min(z,0))
                    am = pe2.tile([P, OC], BF, tag="am")
                    nc.gpsimd.tensor_scalar(am[:], z[:], 0.0, -1.0, OP.max,
                                            OP.add)
                    bm = pe2.tile([P, OC], BF, tag="bm")
                    nc.gpsimd.tensor_scalar(bm[:], z[:], 0.0, None, OP.min)
                    eb = pe2.tile([P, OC], BF, tag="eb")
                    nc.scalar.activation(eb[:], bm[:], AF.Exp)
                    fo = pe2.tile([P, OC], BF, tag="fo")
                    nc.vector.tensor_add(fo[:], am[:], eb[:])
                    nc.sync.dma_start(outT[w * P:(w + 1) * P, :], fo[:])

                # software-pipelined window stages: each engine's
                # in-order queue interleaves adjacent windows
                st1 = {}   # w -> (sc-dependent) emax tile
                st2 = {}   # w -> Wv tile
                st3 = {}   # w -> segp psum tile

                def stage1(w):
                    G = glos[w]
                    sc = pe2.tile([P, EC, NH], F32, tag="sc")
                    nc.vector.tensor_tensor(sc[:, 0:KLO], st_sb[:, w, 0:KLO],
                                            G[:, :, OC:OC + NH], OP.add)
                    nc.vector.tensor_tensor(sc[:, KLO:EC],
                                            st_sb[:, w, KLO:EC],
                                            Ghi[:, w, :, OC:OC + NH], OP.add)
                    e1 = pe2.tile([P, EC, NH], BF, tag="e1")
                    nc.scalar.activation(e1[:], sc[:], AF.Exp)
                    e2 = pe2.tile([P, EC, NH], BF, tag="e2")
                    nc.scalar.activation(e2[:], sc[:], AF.Exp, scale=0.2)
                    emax = pe2.tile([P, EC, NH], BF, tag="emax")
                    nc.vector.tensor_max(emax[:], e1[:], e2[:])
                    st1[w] = emax

                def stage2(w):
                    G = glos[w]
                    emax = st1.pop(w)
                    Wv = pe2.tile([P, EC, 136], BF, tag="Wv")
                    nc.vector.tensor_copy(Wv[:, :, OC:OC + NH], emax[:])
                    emb = emax[:].broadcast_to([P, EC, NH, HD])
                    pool = pmtw if w % 2 == 0 else pbc
                    tag = "Mtw" if w % 2 == 0 else "pbcw"
                    eex = pool.tile([P, cap], BF, tag=tag)
                    ex3 = eex[:].rearrange("p (j f) -> p j f", f=P)
                    nc.scalar.activation(
                        ex3.rearrange("p j (a d) -> p j a d", d=HD),
                        emb, AF.Copy)
                    nc.vector.tensor_tensor(Wv[:, 0:KLO, 0:OC],
                                            G[:, :, 0:OC],
                                            ex3[:, 0:KLO], OP.mult)
                    nc.vector.tensor_tensor(Wv[:, KLO:EC, 0:OC],
                                            Ghi[:, w, :, 0:OC],
                                            ex3[:, KLO:EC], OP.mult)
                    st2[w] = Wv

                def stage3(w):
                    Wv = st2.pop(w)
                    segp = pse.tile([P, 136], F32, tag="segp")
                    for j in range(EC):
                        nc.tensor.matmul(segp[:],
                                         lhsT=Mw[:, (w * EC + j) * P:
                                                 (w * EC + j + 1) * P],
                                         rhs=Wv[:, j, :], start=(j == 0),
                                         stop=(j == EC - 1))
                    st3[w] = segp

                def finalize(w):
                    segp = st3.pop(w)
                    den = pe2.tile([P, NH], F32, tag="den")
                    nc.vector.tensor_scalar_add(den[:], segp[:, OC:OC + NH],
                                                EPS)
                    rec = pe2.tile([P, NH], F32, tag="rec")
                    nc.vector.reciprocal(rec[:], den[:])
                    z = pe2.tile([P, OC], F32, tag="z")
                    recb = rec[:].broadcast_to([P, NH, HD])
                    nc.vector.tensor_tensor(
                        z[:].rearrange("p (a d) -> p a d", d=HD),
                        segp[:, 0:OC].rearrange("p (a d) -> p a d", d=HD),
                        recb, OP.mult)
                    nc.gpsimd.tensor_add(z[:], z[:], skips[:, w])
                    # elu: (max(z,0)-1) + exp(min(z,0))
                    am = pe2.tile([P, OC], BF, tag="am")
                    nc.gpsimd.tensor_scalar(am[:], z[:], 0.0, -1.0, OP.max,
                                            OP.add)
                    bm = pe2.tile([P, OC], BF, tag="bm")
                    nc.gpsimd.tensor_scalar(bm[:], z[:], 0.0, None, OP.min)
                    eb = pe2.tile([P, OC], BF, tag="eb")
                    nc.scalar.activation(eb[:], bm[:], AF.Exp)
                    fo = pe2.tile([P, OC], BF, tag="fo")
                    nc.vector.tensor_add(fo[:], am[:], eb[:])
                    nc.sync.dma_start(outT[w * P:(w + 1) * P, :], fo[:])

                for w in range(W + 3):
                    if w < W:
                        stage1(w)
                    if 1 <= w <= W:
                        stage2(w - 1)
                    if 2 <= w <= W + 1:
                        stage3(w - 2)
                    if 3 <= w:
                        finalize(w - 3)

                # final k-row gather
                ko = pko.tile([P, TP // P, OC], BF, tag="ko")
                nc.gpsimd.dma_gather(ko[:], outT[:], kidxs[:], TP, TP, OC,
                                     single_packet=False)
                nc.sync.dma_start(
                    out[:].rearrange("(j p) f -> p j f", p=P), ko[:])

    nc.compile()
    _BUILD_CACHE[key] = nc
    return nc


# ----------------------------------------------------------------------------
# entry point
# ----------------------------------------------------------------------------

def kernel(x, adj0, index0, w_in, b_in, w_proj, a_src, a_trg, w_skip):
    from concourse.bass_utils import run_bass_kernel_spmd

    per_core, NPAD, EC, KLO, B1 = plan(x, adj0, index0)
    wts = make_weights(w_in, b_in, w_proj, a_src, a_trg, w_skip)
    nc = build(NPAD, EC, KLO, B1)

    in_maps = []
    for c in range(CORES):
        pc = per_core[c]
        fblob = np.concatenate(
            [wts["b1"], wts["iota_c"], pc["etcol"]], axis=1).astype(np.float32)
        in_maps.append(dict(bfblob=wts["bfblob"], fblob=fblob,
                            xTi=pc["xTi"], iblob=pc["iblob"],
                            etrow=pc["etrow"]))

    res = run_bass_kernel_spmd(nc, in_maps, core_ids=list(range(CORES)))
    outs = [r["out"][:KC] for r in res.results]
    return np.concatenate(outs, axis=0).astype(np.float32)
